# revision 28
# baseline (speedup 1.0000x reference)
"""CrossConv2d (concat -> 3x3 conv -> BN -> +skip -> ReLU) on 8 Trainium2 cores.

Data-parallel over the fused (b*s)=32 batch axis: 4 images per core.

1D Winograd F(2,3) along x, direct 3-tap accumulation along y, bf16:
  - host side: inputs are padded, cast to bf16, and x-transformed into
    4 Winograd components D[k]; the BN-scale-folded conv weights become
    12 component matrices G[k][ky] ([Cin, Cout], bf16)
  - the residual skip-add costs nothing: xy_even = (D1 - D2)/2 and
    xy_odd = (D1 + D2)/2 at the middle y-tap, so folding +0.5*I into
    G[1][ky=1] and -0.5*I into G[2][ky=1] makes both output chains
    absorb the skip inside the existing matmuls
  - device side per output chunk (8 rows x 128 cols = 2x512):
      E  = sum_ky G[0][ky]^T D[0]     M1 = sum_ky G[1]'[ky]^T D[1]
      O  = sum_ky -G[3][ky]^T D[3]    M2 = sum_ky G[2]'[ky]^T D[2]
    ScalarE: m1, m2 = copy(M1), copy(M2) to SBUF (PSUM has one DVE read
      port, so downstream ops each read at most one PSUM operand)
    VectorE: t1 = E + m1; u1 = O + m1; od = u1 - m2
    GpSimdE: ev = t1 + m2 (SBUF-only; GpSimd has no PSUM port)
    ScalarE/GpSimdE: og = relu(x + bn_shift) -> bf16, out-DMA per 4 chunks
  - 12 N=512 matmuls per chunk vs 18 for direct conv (1.5x fewer PE
    columns, no padded-width junk); bf16 weights take the fast weight
    load path so LDWEIGHTS hides under the matmul stream
  - DMA is co-limiting: only D (+weights) ever crosses HBM -> ~38MB
    total; D's u half loads once (single D tile, per-image v-half
    overwrites with band-granular deps), outputs batch 4 chunks per DMA
  - output is written as [even 8x64 | odd 8x64] blocks and re-interleaved
    host-side
"""

import numpy as np
import ml_dtypes

import concourse.bacc as bacc
import concourse.mybir as mybir
from concourse import tile
from concourse.bass_utils import run_bass_kernel_spmd

EPS = 1e-5

B, S, C1, C2, H, W = 4, 8, 64, 64, 128, 128
CC = C1 + C2               # 128 concat channels = out channels = partitions
N_CORES = 8
IMG_PER_CORE = (B * S) // N_CORES  # 4
HP = H + 2                 # padded rows (Winograd D spans all 130)
TW = W // 2                # 64 x-tiles per row
NB = 4                     # DMA bands per image
BR = H // NB               # 32 output rows per band
NQ = 16                    # chunks per image (8 rows each)
QR = H // NQ               # 8 rows per chunk

F32 = mybir.dt.float32
BF = mybir.dt.bfloat16
NPBF = ml_dtypes.bfloat16

_CACHE = {}


def _build_program():
    nc = bacc.Bacc(
        "TRN2", target_bir_lowering=False, debug=False, num_devices=N_CORES
    )
    # D components: [ch, comp, padded_row, xtile]; u half shared per core
    du_d = nc.dram_tensor("du", [C1, 4, HP, TW], BF, kind="ExternalInput")
    dv_d = nc.dram_tensor("dv", [IMG_PER_CORE, C2, 4, HP, TW], BF, kind="ExternalInput")
    w_d = nc.dram_tensor("w", [CC, 12 * CC], BF, kind="ExternalInput")
    sh_d = nc.dram_tensor("shift", [CC, 1], F32, kind="ExternalInput")
    # out: per chunk [even 8x64 | odd 8x64]; host re-interleaves
    o_d = nc.dram_tensor("o", [IMG_PER_CORE, CC, NQ * 2 * QR * TW], BF,
                         kind="ExternalOutput")

    with tile.TileContext(nc) as tc:
        with (
            tc.tile_pool(name="consts", bufs=1) as cpool,
            tc.tile_pool(name="scratch", bufs=3) as spool,
            tc.tile_pool(name="og4", bufs=3) as o4pool,
            tc.tile_pool(name="og1", bufs=4) as o1pool,
            tc.tile_pool(name="psum", bufs=8, space="PSUM") as ppool,
        ):
            w_r = cpool.tile([CC, 12 * CC], BF)
            # weight pieces in first-use order (k=1,2 stream first)
            nc.scalar.dma_start(w_r[:, 3 * CC : 9 * CC], w_d[:, 3 * CC : 9 * CC])
            sh_sb = cpool.tile([CC, 1], F32)
            nc.scalar.dma_start(sh_sb[:], sh_d[:])
            nc.scalar.dma_start(w_r[:, 0 : 3 * CC], w_d[:, 0 : 3 * CC])
            nc.scalar.dma_start(w_r[:, 9 * CC : 12 * CC], w_d[:, 9 * CC : 12 * CC])

            # single whole-image D tile: u half loaded once; v half
            # overwritten per image in bands (deps are band-granular, so
            # image i+1's loads overlap image i's tail compute)
            d_t = cpool.tile([CC, 4, HP, TW], BF)

            # band row ranges in padded-D space; image 0 sliced finer so
            # the first chunks' operands land quickly
            def d_bands(first_img):
                if first_img:
                    cuts = [0, QR + 2, 2 * QR + 2, 3 * QR + 2, BR + 2,
                            BR + 2 + 16, 2 * BR + 2, 2 * BR + 2 + 16,
                            3 * BR + 2, 3 * BR + 2 + 16, HP]
                    return list(zip(cuts, cuts[1:]))
                return [(bnd * BR + 2 if bnd else 0, (bnd + 1) * BR + 2)
                        for bnd in range(NB)]

            for img in range(IMG_PER_CORE):
                first = img == 0
                # v half on the sync queue; u half (image 0 only) as a
                # parallel issue stream on scalar, ahead of any compute
                for dr0, dr1 in d_bands(first):
                    nc.sync.dma_start(
                        d_t[C1:CC, :, dr0:dr1, :], dv_d[img, :, :, dr0:dr1, :]
                    )
                    if first:
                        nc.scalar.dma_start(
                            d_t[0:C1, :, dr0:dr1, :], du_d[:, :, dr0:dr1, :]
                        )

                last_img = img == IMG_PER_CORE - 1
                og4 = None
                # software-pipelined epilogue: chunk c's even-relu and the
                # group's out-DMA are emitted during chunk c+1, so V never
                # sits waiting on GpSimd's ev inside its in-order queue
                pend = None  # (oge_ap, ev_tile, dma_emit_fn)

                def flush_pend():
                    nonlocal pend
                    if pend is None:
                        return
                    oge_p, ev_p, dma_fn = pend
                    nc.vector.tensor_scalar_max(oge_p.opt(), ev_p[:], 0.0)
                    if dma_fn is not None:
                        dma_fn()
                    pend = None

                for bnd in range(NB):
                    for q in range(NB):
                        rg = bnd * BR + q * QR      # global output row
                        ci = bnd * NB + q
                        # E and O share one 2-bank PSUM tile so one wide
                        # stt handles both chains below
                        ps_eo = ppool.tile([CC, 2, 512], F32, tag="eo",
                                           bufs=2)
                        psm1 = ppool.tile([CC, 512], F32, tag="m1p", bufs=2)
                        psm2 = ppool.tile([CC, 512], F32, tag="m2p", bufs=2)
                        outs = {0: ps_eo[:, 0], 1: psm1[:], 2: psm2[:],
                                3: ps_eo[:, 1]}
                        # M1/M2 first so the scalar copies start early
                        for k in (1, 2, 0, 3):
                            for ky in range(3):
                                nc.tensor.matmul(
                                    outs[k],
                                    w_r[:, (3 * k + ky) * CC : (3 * k + ky + 1) * CC],
                                    d_t[:, k, rg + ky : rg + ky + QR, :],
                                    start=(ky == 0),
                                    stop=(ky == 2),
                                )
                        flush_pend()
                        m1 = spool.tile([CC, 512], F32, tag="m1")
                        m2 = spool.tile([CC, 512], F32, tag="m2")
                        nc.scalar.copy(m1[:], psm1[:])
                        nc.scalar.copy(m2[:], psm2[:])
                        tu = spool.tile([CC, 2, 512], F32, tag="tu")
                        od = spool.tile([CC, 512], F32, tag="od")
                        ev = spool.tile([CC, 512], F32, tag="ev")
                        # one wide op for both chains, BN shift folded in:
                        # tu = (ps_eo + shift) + m1 (broadcast across halves)
                        m1b = m1[:].unsqueeze(1).broadcast_to([CC, 2, 512])
                        nc.vector.scalar_tensor_tensor(
                            tu[:], ps_eo[:], sh_sb[:], m1b,
                            mybir.AluOpType.add, mybir.AluOpType.add,
                        )
                        # SBUF-only ops go to GpSimd (no PSUM port there);
                        # od alternates V/G to balance queue load; the
                        # final chunk runs entirely on V (shorter tail)
                        tail_chunk = last_img and ci == NQ - 1
                        if tail_chunk:
                            nc.vector.tensor_add(ev[:], tu[:, 0], m2[:])
                        else:
                            nc.gpsimd.tensor_add(ev[:], tu[:, 0], m2[:])
                        if ci % 2 == 0 and not tail_chunk:
                            nc.gpsimd.tensor_sub(od[:], tu[:, 1], m2[:])
                        else:
                            nc.vector.tensor_sub(od[:], tu[:, 1], m2[:])
                        if last_img:
                            og = o1pool.tile([CC, 2, QR, TW], BF, tag="og1")
                            oge, ogo = og[:, 0], og[:, 1]
                        else:
                            if ci % 4 == 0:
                                og4 = o4pool.tile([CC, 4, 2, QR, TW], BF,
                                                  tag="og4")
                            oge, ogo = og4[:, ci % 4, 0], og4[:, ci % 4, 1]
                        # odd relu on scalar (only the copies live there)
                        nc.scalar.activation(
                            ogo.opt(), od[:],
                            mybir.ActivationFunctionType.Relu,
                        )
                        # outputs on gpsimd so the sync queue only carries
                        # input loads (image i+1's D issues must not sit
                        # behind image i's output issues)
                        if last_img:
                            def dma_fn(img=img, ci=ci, og=og):
                                nc.sync.dma_start(
                                    o_d[img, :, ci * 1024 : (ci + 1) * 1024],
                                    og[:, :, :, :],
                                )
                        elif ci % 4 == 3:
                            def dma_fn(img=img, ci=ci, og4=og4):
                                nc.gpsimd.dma_start(
                                    o_d[img, :, (ci - 3) * 1024 : (ci + 1) * 1024],
                                    og4[:, :, :, :, :],
                                )
                        else:
                            dma_fn = None
                        pend = (oge, ev, dma_fn)
                        if tail_chunk:
                            flush_pend()
                flush_pend()
    nc.compile()
    return nc


def _get_program():
    if "nc" not in _CACHE:
        _CACHE["nc"] = _build_program()
    return _CACHE["nc"]


def _prep_inputs(u, v, conv_w, bn_gamma, bn_beta, bn_mean, bn_var):
    u = np.asarray(u, dtype=np.float32)
    v = np.asarray(v, dtype=np.float32)
    conv_w = np.asarray(conv_w, dtype=np.float32)
    bn_gamma = np.asarray(bn_gamma, dtype=np.float32)
    bn_beta = np.asarray(bn_beta, dtype=np.float32)
    bn_mean = np.asarray(bn_mean, dtype=np.float32)
    bn_var = np.asarray(bn_var, dtype=np.float32)

    scale = bn_gamma / np.sqrt(bn_var + EPS)
    shift = (bn_beta - bn_mean * scale).astype(np.float32).reshape(CC, 1)
    wsc = conv_w * scale[:, None, None, None]  # [out, in, ky, kx]
    W0, W1, W2 = wsc[..., 0], wsc[..., 1], wsc[..., 2]  # [out, in, ky]
    G = [W0, (W0 + W1 + W2) * 0.5, (W0 - W1 + W2) * 0.5, -W2]
    # fold the skip-add: xy_even = (D1 - D2)/2, xy_odd = (D1 + D2)/2 at
    # the middle y-tap (both output chains then absorb it exactly)
    eye = np.eye(CC, dtype=np.float32)
    G[1] = G[1].copy()
    G[2] = G[2].copy()
    G[1][:, :, 1] += 0.5 * eye
    G[2][:, :, 1] -= 0.5 * eye
    w_host = np.zeros((CC, 12 * CC), np.float32)
    for k in range(4):
        for ky in range(3):
            # lhsT block [in, out]
            w_host[:, (3 * k + ky) * CC : (3 * k + ky + 1) * CC] = G[k][:, :, ky].T
    w_host = w_host.astype(NPBF)

    def transform(x):
        """x: [C, H, W] fp32 -> D [C,4,HP,TW] bf16."""
        C = x.shape[0]
        p = np.zeros((C, HP, W + 2), np.float32)
        p[:, 1 : 1 + H, 1 : 1 + W] = x
        p = p.astype(NPBF).astype(np.float32)
        D = np.empty((C, 4, HP, TW), np.float32)
        D[:, 0] = p[:, :, 0 : 2 * TW : 2] - p[:, :, 2 : 2 * TW + 2 : 2]
        D[:, 1] = p[:, :, 1 : 2 * TW + 1 : 2] + p[:, :, 2 : 2 * TW + 2 : 2]
        D[:, 2] = p[:, :, 2 : 2 * TW + 2 : 2] - p[:, :, 1 : 2 * TW + 1 : 2]
        D[:, 3] = p[:, :, 1 : 2 * TW + 1 : 2] - p[:, :, 3 : 2 * TW + 3 : 2]
        return D.astype(NPBF)

    in_maps = []
    du_cache = {}
    for m in range(N_CORES):
        b = m // 2
        s0 = (m % 2) * IMG_PER_CORE
        if b not in du_cache:
            du_cache[b] = transform(u[b, 0])
        du = du_cache[b]
        dv = np.empty((IMG_PER_CORE, C2, 4, HP, TW), NPBF)
        for i in range(IMG_PER_CORE):
            dv[i] = transform(v[b, s0 + i])
        in_maps.append({"du": du, "dv": dv, "w": w_host, "shift": shift})
    return in_maps


def _run(inputs, trace=False):
    nc = _get_program()
    in_maps = _prep_inputs(**inputs)
    res = run_bass_kernel_spmd(nc, in_maps, list(range(N_CORES)), trace=trace)
    out = np.empty((B, 1, S, CC, H, W), np.float32)
    for m in range(N_CORES):
        b = m // 2
        s0 = (m % 2) * IMG_PER_CORE
        o = np.asarray(res.results[m]["o"]).astype(np.float32)
        # [img, CC, chunk, eo, row, xtile] -> [img, CC, chunk*row, xtile*2+eo]
        o = o.reshape(IMG_PER_CORE, CC, NQ, 2, QR, TW)
        o = o.transpose(0, 1, 2, 4, 5, 3).reshape(IMG_PER_CORE, CC, H, W)
        out[b, 0, s0 : s0 + IMG_PER_CORE] = o
    return out, res


def kernel(u, v, conv_w, bn_gamma, bn_beta, bn_mean, bn_var):
    out, _ = _run(
        dict(u=u, v=v, conv_w=conv_w, bn_gamma=bn_gamma, bn_beta=bn_beta,
             bn_mean=bn_mean, bn_var=bn_var)
    )
    return out
